# revision 4
# baseline (speedup 1.0000x reference)
"""KAN 3x3 convolution kernel for 8 Trainium2 NeuronCores.

Math: out[b,o,ih,iw] = sum_{c,k} scale_base[o,c,k]*silu(t) + sum_{c,k,m} W_sp[o,c,k,m]*B3_m(t)
where t = xpad[b,c,ih+di,iw+dj] for kernel position k=(di,dj), and B3_m is the
cubic B-spline basis on the uniform extended grid [-2.2, 2.2], h=0.4.

Key identity (reflection form, bounded intermediates so f32r rounding is benign):
  B3_m(t) = N3(u), u = 2.5t + 5.5 - m,  z = |u - 2|
  N3 = (1/6) * ( relu(2 - z)^3 - 4*relu(1 - z)^3 )
Features: 8 bounded B3 values + silu(t) + const 1 per (c, pixel); the whole op is
a linear contraction per kernel position -> shift-and-matmul (1/6 folded into
weights on the host).

Per core: one batch element. Features computed once per pixel (34x34 padded grid,
4 m-blocks stacked in partitions per chunk), then 54 accumulating f32r matmuls
(9 kernel positions x 3 contraction chunks x 2 column blocks of 512 pixels).
"""
import sys
sys.path.insert(0, '/opt/trn_rl_repo')
import warnings
warnings.filterwarnings('ignore')
import numpy as np

import concourse.bass as bass
import concourse.mybir as mybir
import concourse.tile as tile
from concourse.bass_utils import run_bass_kernel_spmd

B, C, O, H, W = 8, 32, 64, 32, 32
KH = KW = 3
NUM, KS = 5, 3
M = NUM + KS            # 8 spline bases
NFEAT = 12              # relu^3 features
HP = WP = 34            # padded grid
NPIX = HP * WP          # 1156
DT_MM = mybir.dt.float32r


class _TC(tile.TileContext):
    """TileContext whose final drain splits sem waits to <=2 per instruction
    (walrus CTRL codegen rejects drains with too many sync waits)."""

    def _drain_and_barrier(self, tick_clock, wait_clock):
        from concourse.vector_clock import ScopedClock
        nc = self.nc
        drain_inst = nc.sync.drain()
        wait_clock.add_sem_waits(
            drain_inst.ins, ScopedClock({None: tick_clock.global_clock})
        )
        si = drain_inst.ins.sync_info
        waits = list(si.on_wait or [])
        MAXW = 1
        if len(waits) > MAXW:
            del si.on_wait[MAXW:]
            rest = waits[MAXW:]
            for i in range(0, len(rest), MAXW):
                d2 = nc.sync.drain()
                s2 = d2.ins.sync_info
                if s2 is None:
                    s2 = type(si)(on_wait=[], on_update=[])
                    d2.ins.sync_info = s2
                s2.on_wait.extend(rest[i:i + MAXW])
        nc.all_engine_barrier()
        popped = nc._tile_sem_poison_stack.pop()
        assert popped is self._sem_poison
        nc.clear_and_free_semaphores(list(self.sems.allocated().values()))
        nc.all_engine_barrier()


def _host_weights(coef, scale_base, scale_sp, bias):
    """Fold scale_sp and the 1/6 into lhsT weights [128, 9, 3, 64]
    (chunk j in {0,1}: rows = 32*m_local + c hold W_sp[o,c,k,4j+ml]/6;
    chunk 2: silu rows + const/bias row)."""
    W_sp = (scale_sp[..., None] * coef).astype(np.float32)   # (O, C, 9, M)
    wfull = np.zeros((128, KH * KW, 3, O), dtype=np.float32)
    for j in range(2):
        for ml in range(4):
            m = 4 * j + ml
            wfull[32 * ml:32 * ml + 32, :, j, :] = (
                W_sp[:, :, :, m] / 6.0).transpose(1, 2, 0)
    wfull[0:32, :, 2, :] = scale_base.transpose(1, 2, 0)     # (c, k, o)
    wfull[32, 4, 2, :] = bias
    return wfull


def _build_nc():
    nc = bass.Bass()
    # x (4 replica blocks) with the 2 per-partition abs-bias columns appended,
    # so every feature op has a single-producer dependency (1 sync wait max).
    x_d = nc.dram_tensor("xpad", [128, NPIX + 4], mybir.dt.float32,
                         kind="ExternalInput")
    w_d = nc.dram_tensor("wfull", [128, KH * KW * 3 * O], mybir.dt.float32,
                         kind="ExternalInput")
    o_d = nc.dram_tensor("out", [O, H * W], mybir.dt.float32, kind="ExternalOutput")

    with _TC(nc) as tc:
        with tc.tile_pool(name="sb", bufs=1) as sb, \
             tc.tile_pool(name="eps", bufs=2) as eps, \
             tc.tile_pool(name="ps", bufs=2, space="PSUM") as ps:
            xb = sb.tile([128, NPIX + 4], mybir.dt.float32)
            nc.sync.dma_start(out=xb[:, :], in_=x_d[:, :])
            xq = xb[:, 0:NPIX].rearrange("p (h w) -> p h w", h=HP)

            wf = sb.tile([128, KH * KW * 3 * O], mybir.dt.float32)
            nc.sync.dma_start(out=wf[:, :], in_=w_d[:, :])
            # cast weights to f32r on ACT (first ACT op): the j2-first matmul
            # then depends only on ACT-produced operands (1 sync wait).
            wt = sb.tile([128, KH * KW, 3, O], DT_MM)
            nc.scalar.activation(
                wt[:, :, :, :],
                wf[:, :].rearrange("p (k j o) -> p k j o", k=KH * KW, j=3),
                mybir.ActivationFunctionType.Copy)

            # ---- features: B3 via reflection identity, all bounded ----
            # silu chunk first so its matmuls can start early; chunk0 muls on
            # DVE, chunk1 squares on ACT + cubes on Pool to balance engines
            # while keeping every instruction at <=1 cross-engine wait.
            AF = mybir.ActivationFunctionType
            c3 = sb.tile([33, HP, WP], DT_MM)
            nc.scalar.activation(c3[0:32, :, :], xq[0:32, :, :], AF.Silu)
            nc.scalar.activation(c3[32:33, :, :], xq[0:1, :, :], AF.Relu,
                                 bias=xb[0:1, NPIX + 3:NPIX + 4], scale=0.0)
            chunks = []
            for jc in range(2):
                z = sb.tile([128, HP, WP], mybir.dt.float32, tag=f"z{jc}")
                nc.scalar.activation(z[:, :, :], xq, AF.Abs,
                                     bias=xb[:, NPIX + jc:NPIX + jc + 1], scale=2.5)
                p = sb.tile([128, HP, WP], mybir.dt.float32, tag=f"p{jc}")
                nc.scalar.activation(p[:, :, :], z[:, :, :], AF.Relu,
                                     bias=xb[:, NPIX + 2:NPIX + 3], scale=-1.0)
                q = sb.tile([128, HP, WP], mybir.dt.float32, tag=f"q{jc}")
                nc.scalar.activation(q[:, :, :], z[:, :, :], AF.Relu,
                                     bias=xb[:, NPIX + 3:NPIX + 4], scale=-1.0)
                psq = sb.tile([128, HP, WP], mybir.dt.float32, tag=f"psq{jc}")
                qsq = sb.tile([128, HP, WP], mybir.dt.float32, tag=f"qsq{jc}")
                pc = sb.tile([128, HP, WP], mybir.dt.float32, tag=f"pc{jc}")
                qc = sb.tile([128, HP, WP], mybir.dt.float32, tag=f"qc{jc}")
                if jc == 0:
                    nc.vector.tensor_mul(psq[:, :, :], p[:, :, :], p[:, :, :])
                    nc.vector.tensor_mul(pc[:, :, :], psq[:, :, :], p[:, :, :])
                    nc.vector.tensor_mul(qsq[:, :, :], q[:, :, :], q[:, :, :])
                    nc.vector.tensor_mul(qc[:, :, :], qsq[:, :, :], q[:, :, :])
                else:
                    nc.scalar.activation(psq[:, :, :], p[:, :, :], AF.Square)
                    nc.scalar.activation(qsq[:, :, :], q[:, :, :], AF.Square)
                    nc.gpsimd.tensor_mul(pc[:, :, :], psq[:, :, :], p[:, :, :])
                    nc.gpsimd.tensor_mul(qc[:, :, :], qsq[:, :, :], q[:, :, :])
                cb = sb.tile([128, HP, WP], DT_MM, tag=f"cb{jc}")
                # cb = pc - 4*qc  (bounded <= ~1.33; rounded to f32r here)
                nc.vector.scalar_tensor_tensor(
                    out=cb[:, :, :], in0=qc[:, :, :], scalar=-4.0,
                    in1=pc[:, :, :], op0=mybir.AluOpType.mult,
                    op1=mybir.AluOpType.add)
                chunks.append(cb)
            chunks.append(c3)

            # ---- 54 accumulating matmuls + epilogue ----
            # j order: silu chunk (ready first), then cb0 (DVE), then cb1.
            rows = [128, 128, 33]
            jorder = [2, 0, 1]
            for lc in range(2):
                psum = ps.tile([O, 512], mybir.dt.float32)
                first = True
                for j in jorder:
                    for k in range(KH * KW):
                        ik, jk = k // 3, k % 3
                        r = rows[j]
                        rhs = chunks[j][0:r, lc * 16 + ik: lc * 16 + ik + 16,
                                        jk: jk + 32]
                        nc.tensor.matmul(
                            psum[:, :], wt[0:r, k, j, :], rhs,
                            start=first, stop=(j == jorder[-1] and k == KH * KW - 1))
                        first = False
                ot = eps.tile([O, 512], mybir.dt.float32, tag="ot")
                nc.vector.tensor_copy(ot[:, :], psum[:, :])
                nc.sync.dma_start(out=o_d[:, 512 * lc: 512 * (lc + 1)], in_=ot[:, :])
    return nc


_BVEC = np.zeros((128, 4), dtype=np.float32)
for _j in range(2):
    for _q in range(4):
        _BVEC[32 * _q:32 * _q + 32, _j] = 3.5 - (4 * _j + _q)
_BVEC[:, 2] = 2.0
_BVEC[:, 3] = 1.0

_NC_CACHE = {}


def _get_nc():
    if "nc" not in _NC_CACHE:
        _NC_CACHE["nc"] = _build_nc()
    return _NC_CACHE["nc"]


def _run(x, coef, scale_base, scale_sp, bias, trace=False):
    nc = _get_nc()
    kw = {}
    if trace:
        import os
        td = os.environ.get("KAN_TRACE_DIR")
        if td:
            os.makedirs(td, exist_ok=True)
            kw["tmpdir"] = td
    wfull = _host_weights(np.asarray(coef), np.asarray(scale_base),
                          np.asarray(scale_sp), np.asarray(bias))
    wflat = np.ascontiguousarray(wfull.reshape(128, -1))
    x = np.asarray(x)
    in_maps = []
    for b in range(B):
        xpad = np.zeros((C, HP, WP), dtype=np.float32)
        xpad[:, 1:1 + H, 1:1 + W] = x[b]
        x4 = np.tile(xpad.reshape(C, NPIX), (4, 1))
        xb = np.concatenate([x4, _BVEC], axis=1)
        in_maps.append({"xpad": np.ascontiguousarray(xb), "wfull": wflat})
    res = run_bass_kernel_spmd(nc, in_maps, core_ids=list(range(B)), trace=trace,
                               **kw)
    out = np.stack([res.results[b]["out"].reshape(O, H, W) for b in range(B)])
    return out, res


def kernel(x, coef, scale_base, scale_sp, bias):
    out, _ = _run(x, coef, scale_base, scale_sp, bias, trace=False)
    return out


def kernel_traced(x, coef, scale_base, scale_sp, bias):
    # test.py injects the NTFF hook (antenv.axon_hooks) before importing us;
    # if absent, bass_utils degrades to untraced gracefully.
    out, res = _run(x, coef, scale_base, scale_sp, bias, trace=True)
    return out, res



# revision 14
# speedup vs baseline: 1.3997x; 1.3997x over previous
"""KAN 3x3 convolution kernel for 8 Trainium2 NeuronCores.

Math: out[b,o,ih,iw] = sum_{c,k} scale_base[o,c,k]*silu(t) + sum_{c,k,m} W_sp[o,c,k,m]*B3_m(t)
where t = xpad[b,c,ih+di,iw+dj] for kernel position k=(di,dj), and B3_m is the
cubic B-spline basis on the uniform extended grid [-2.2, 2.2], h=0.4.

Reflection identity with constants folded so the combine is a plain subtract:
  B3_m(t) = (P^3 - Q^3)/12,  z = |2.5t + 3.5 - m|
  P = relu(2c - c*z) (c = 2^{1/3}),  Q = relu(2 - 2z)
All features bounded (P<=2.52, Q<=2) so bf16 rounding is benign; matmuls in
bf16 (1 col/cycle on the PE, 2x DVE elementwise).

Per core: one batch element. 21 weight groups x 2 column blocks of 512 pixels:
  groups 0-2:  silu chunk, 3 vertical shifts baked into partitions (97 rows
               incl. const-1 bias row) -> only the horizontal shift dj remains
               in the rhs view (3 matmuls instead of 9)
  groups 3-20: two spline chunks (4 m-values x 32 channels = 128 rows) x 9
               kernel positions via shifted rhs views
Weight matrices ping-pong between PE column tiles (0,0)/(0,64) so loads hide
under streaming; each group's second matmul reuses the loaded weights
(ldweights=False). psum halves are summed in the epilogue.
"""
import sys
sys.path.insert(0, '/opt/trn_rl_repo')
import warnings
warnings.filterwarnings('ignore')
import numpy as np
import ml_dtypes

import concourse.bass as bass
import concourse.mybir as mybir
import concourse.tile as tile
from concourse.bass_utils import run_bass_kernel_spmd

B, C, O, H, W = 8, 32, 64, 32, 32
KH = KW = 3
NUM, KS = 5, 3
M = NUM + KS            # 8 spline bases
HP = WP = 34            # padded grid
NPIX = HP * WP          # 1156
NG = 21                 # weight groups
DT = mybir.dt.bfloat16
CBRT2 = 2.0 ** (1.0 / 3.0)

USE_TILEPOS = True      # ping-pong weight tiles (0,0)/(0,64)
USE_LDW_SKIP = True     # reuse loaded weights for the 2nd column block


class _TC(tile.TileContext):
    """TileContext whose final drain splits sem waits to <=2 per instruction
    (walrus CTRL codegen rejects drains with too many sync waits)."""

    def _drain_and_barrier(self, tick_clock, wait_clock):
        from concourse.vector_clock import ScopedClock
        nc = self.nc
        drain_inst = nc.sync.drain()
        wait_clock.add_sem_waits(
            drain_inst.ins, ScopedClock({None: tick_clock.global_clock})
        )
        si = drain_inst.ins.sync_info
        waits = list(si.on_wait or [])
        MAXW = 1
        if len(waits) > MAXW:
            del si.on_wait[MAXW:]
            rest = waits[MAXW:]
            for i in range(0, len(rest), MAXW):
                d2 = nc.sync.drain()
                s2 = d2.ins.sync_info
                if s2 is None:
                    s2 = type(si)(on_wait=[], on_update=[])
                    d2.ins.sync_info = s2
                s2.on_wait.extend(rest[i:i + MAXW])
        nc.all_engine_barrier()
        popped = nc._tile_sem_poison_stack.pop()
        assert popped is self._sem_poison
        nc.clear_and_free_semaphores(list(self.sems.allocated().values()))
        nc.all_engine_barrier()


def _host_weights(coef, scale_base, scale_sp, bias):
    """bf16 lhsT weights [128, 21, 64]: groups 0-2 silu (rows 32*di+c,
    const/bias row 96 in group dj=1), groups 3+9*jc+k spline (rows 32*ml+c
    hold W_sp[o,c,k,4jc+ml]/12)."""
    W_sp = (scale_sp[..., None] * coef).astype(np.float32)   # (O, C, 9, M)
    wfull = np.zeros((128, NG, O), dtype=np.float32)
    for dj in range(3):
        for di in range(3):
            wfull[32 * di:32 * di + 32, dj, :] = (
                scale_base[:, :, 3 * di + dj].T)             # (c, o)
    wfull[96, 1, :] = bias
    for jc in range(2):
        for k in range(KH * KW):
            for ml in range(4):
                wfull[32 * ml:32 * ml + 32, 3 + 9 * jc + k, :] = (
                    W_sp[:, :, k, 4 * jc + ml].T / 12.0)
    return wfull.astype(ml_dtypes.bfloat16)


def _build_nc():
    nc = bass.Bass()
    # x (4 replica blocks) with 2 per-partition z-bias columns appended.
    x_d = nc.dram_tensor("xpad", [128, NPIX + 4], DT, kind="ExternalInput")
    w_d = nc.dram_tensor("wfull", [128, NG * O], DT, kind="ExternalInput")
    o_d = nc.dram_tensor("out", [O, H * W], mybir.dt.float32, kind="ExternalOutput")

    with _TC(nc) as tc:
        with tc.tile_pool(name="sb", bufs=1) as sb, \
             tc.tile_pool(name="eps", bufs=2) as eps, \
             tc.tile_pool(name="ps", bufs=2, space="PSUM") as ps:
            xb = sb.tile([128, NPIX + 4], DT)
            nc.sync.dma_start(out=xb[:, :], in_=x_d[:, :])
            xq = xb[:, 0:NPIX].rearrange("p (h w) -> p h w", h=HP)

            wf = sb.tile([128, NG * O], DT)
            nc.sync.dma_start(out=wf[:, :], in_=w_d[:, :])

            AF = mybir.ActivationFunctionType
            # ---- silu chunk: rows 32*di+c = silu shifted di rows up; row 96 = 1
            # All F writes go through DVE (silu staged via sl) so matmuls carry
            # exactly one sem wait (walrus MM codegen rejects >1).
            F = sb.tile([97, HP, WP], DT)
            sl = sb.tile([32, HP, WP], DT)
            scr = sb.tile([1, 1], DT)
            # 1-element read of wf on DVE: its wait covers the weights DMA.
            nc.vector.tensor_copy(scr[:, :], wf[0:1, 0:1])
            nc.scalar.activation(sl[:, :, :], xq[0:32, :, :], AF.Silu)
            nc.vector.memset(F[96:97, :, :], 1.0)
            nc.vector.tensor_copy(F[0:32, :, :], sl[:, :, :])
            nc.vector.tensor_copy(F[32:64, 0:32, :], sl[0:32, 1:33, :])
            nc.vector.tensor_copy(F[64:96, 0:32, :], sl[0:32, 2:34, :])

            # ---- spline chunks: cb = P^3 - Q^3 (= 12*B3), all ops bf16
            chunks = []
            for jc in range(2):
                z = sb.tile([128, HP, WP], DT, tag=f"z{jc}")
                nc.scalar.activation(z[:, :, :], xq, AF.Abs,
                                     bias=xb[:, NPIX + jc:NPIX + jc + 1],
                                     scale=2.5)
                P = sb.tile([128, HP, WP], DT, tag=f"P{jc}")
                nc.scalar.activation(P[:, :, :], z[:, :, :], AF.Relu,
                                     bias=xb[:, NPIX + 2:NPIX + 3], scale=-CBRT2)
                Q = sb.tile([128, HP, WP], DT, tag=f"Q{jc}")
                nc.scalar.activation(Q[:, :, :], z[:, :, :], AF.Relu,
                                     bias=xb[:, NPIX + 3:NPIX + 4], scale=-2.0)
                P2 = sb.tile([128, HP, WP], DT, tag=f"P2{jc}")
                nc.vector.tensor_mul(P2[:, :, :], P[:, :, :], P[:, :, :])
                P3 = sb.tile([128, HP, WP], DT, tag=f"P3{jc}")
                nc.vector.tensor_mul(P3[:, :, :], P2[:, :, :], P[:, :, :])
                Q2 = sb.tile([128, HP, WP], DT, tag=f"Q2{jc}")
                nc.vector.tensor_mul(Q2[:, :, :], Q[:, :, :], Q[:, :, :])
                Q3 = sb.tile([128, HP, WP], DT, tag=f"Q3{jc}")
                nc.vector.tensor_mul(Q3[:, :, :], Q2[:, :, :], Q[:, :, :])
                cb = sb.tile([128, HP, WP], DT, tag=f"cb{jc}")
                nc.vector.tensor_sub(cb[:, :, :], P3[:, :, :], Q3[:, :, :])
                chunks.append(cb)

            # ---- 42 accumulating matmuls + epilogue ----
            psum = []
            for lc in range(2):
                pst = ps.tile([128, 512], mybir.dt.float32, tag=f"ps{lc}")
                psum.append(pst)
            for g in range(NG):
                pos = (g % 2) * 64 if USE_TILEPOS else 0
                if g < 3:
                    rows, dj = 97, g
                else:
                    jc, k = (g - 3) // 9, (g - 3) % 9
                    rows, ik, jk = 128, k // 3, k % 3
                for lc in range(2):
                    if g < 3:
                        rhs = F[0:rows, lc * 16: lc * 16 + 16, dj: dj + 32]
                    else:
                        rhs = chunks[jc][0:rows,
                                         lc * 16 + ik: lc * 16 + ik + 16,
                                         jk: jk + 32]
                    start = g < (2 if USE_TILEPOS else 1)
                    stop = g >= NG - (2 if USE_TILEPOS else 1)
                    nc.tensor.matmul(
                        psum[lc][pos: pos + O, :], wf[0:rows, O * g: O * g + O],
                        rhs, start=start, stop=stop,
                        tile_position=(0, pos) if USE_TILEPOS else None)
            for lc in range(2):
                ot = eps.tile([O, 512], mybir.dt.float32, tag="ot")
                if USE_TILEPOS:
                    # DVE-only epilogue (one PSUM operand per op): copy the lo
                    # half (finishes last, so its PE wait covers the hi half),
                    # then the add carries no waits.
                    oh = eps.tile([O, 512], mybir.dt.float32, tag="oh")
                    nc.vector.tensor_copy(oh[:, :], psum[lc][0:O, :])
                    nc.vector.tensor_add(ot[:, :], psum[lc][O:2 * O, :], oh[:, :])
                else:
                    nc.vector.tensor_copy(ot[:, :], psum[lc][0:O, :])
                nc.sync.dma_start(out=o_d[:, 512 * lc: 512 * (lc + 1)],
                                  in_=ot[:, :])
    return nc


_BVEC = np.zeros((128, 4), dtype=np.float32)
for _jc in range(2):
    for _ml in range(4):
        _BVEC[32 * _ml:32 * _ml + 32, _jc] = 3.5 - (4 * _jc + _ml)
_BVEC[:, 2] = 2.0 * CBRT2
_BVEC[:, 3] = 2.0

_NC_CACHE = {}


def _get_nc():
    if "nc" not in _NC_CACHE:
        _NC_CACHE["nc"] = _build_nc()
    return _NC_CACHE["nc"]


def _run(x, coef, scale_base, scale_sp, bias, trace=False):
    nc = _get_nc()
    kw = {}
    if trace:
        import os
        td = os.environ.get("KAN_TRACE_DIR")
        if td:
            os.makedirs(td, exist_ok=True)
            kw["tmpdir"] = td
    wfull = _host_weights(np.asarray(coef), np.asarray(scale_base),
                          np.asarray(scale_sp), np.asarray(bias))
    wflat = np.ascontiguousarray(wfull.reshape(128, -1))
    x = np.asarray(x)
    in_maps = []
    for b in range(B):
        xpad = np.zeros((C, HP, WP), dtype=np.float32)
        xpad[:, 1:1 + H, 1:1 + W] = x[b]
        x4 = np.tile(xpad.reshape(C, NPIX), (4, 1))
        xb = np.concatenate([x4, _BVEC], axis=1).astype(ml_dtypes.bfloat16)
        in_maps.append({"xpad": np.ascontiguousarray(xb), "wfull": wflat})
    res = run_bass_kernel_spmd(nc, in_maps, core_ids=list(range(B)), trace=trace,
                               **kw)
    out = np.stack([res.results[b]["out"].reshape(O, H, W) for b in range(B)])
    return out, res


def kernel(x, coef, scale_base, scale_sp, bias):
    out, _ = _run(x, coef, scale_base, scale_sp, bias, trace=False)
    return out


def kernel_traced(x, coef, scale_base, scale_sp, bias):
    # test.py injects the NTFF hook (antenv.axon_hooks) before importing us;
    # if absent, bass_utils degrades to untraced gracefully.
    out, res = _run(x, coef, scale_base, scale_sp, bias, trace=True)
    return out, res


# revision 16
# speedup vs baseline: 1.5027x; 1.0735x over previous
"""KAN 3x3 convolution kernel for 8 Trainium2 NeuronCores.

Math: out[b,o,ih,iw] = sum_{c,k} scale_base[o,c,k]*silu(t) + sum_{c,k,m} W_sp[o,c,k,m]*B3_m(t)
where t = xpad[b,c,ih+di,iw+dj] for kernel position k=(di,dj), and B3_m is the
cubic B-spline basis on the uniform extended grid [-2.2, 2.2], h=0.4.

Reflection identity with constants folded so the combine is a plain subtract:
  B3_m(t) = (P^3 - Q^3)/12,  z = |2.5t + 3.5 - m|
  P = relu(2c - c*z) (c = 2^{1/3}),  Q = relu(2 - 2z)
All features bounded (P<=2.52, Q<=2) so bf16 rounding is benign; matmuls in
bf16 (1 col/cycle on the PE, 2x DVE elementwise).

Per core: one batch element. 21 weight groups x 2 column blocks of 512 pixels:
  groups 0-2:  silu chunk, 3 vertical shifts baked into partitions (97 rows
               incl. const-1 bias row) -> only the horizontal shift dj remains
               in the rhs view (3 matmuls instead of 9)
  groups 3-20: two spline chunks (4 m-values x 32 channels = 128 rows) x 9
               kernel positions via shifted rhs views
Weight matrices ping-pong between PE column tiles (0,0)/(0,64) so loads hide
under streaming; each group's second matmul reuses the loaded weights
(ldweights=False). psum halves are summed in the epilogue.
"""
import sys
sys.path.insert(0, '/opt/trn_rl_repo')
import warnings
warnings.filterwarnings('ignore')
import numpy as np
import ml_dtypes

import concourse.bass as bass
import concourse.mybir as mybir
import concourse.tile as tile
from concourse.bass_utils import run_bass_kernel_spmd

B, C, O, H, W = 8, 32, 64, 32, 32
KH = KW = 3
NUM, KS = 5, 3
M = NUM + KS            # 8 spline bases
HP = WP = 34            # padded grid
NPIX = HP * WP          # 1156
NG = 21                 # weight groups
DT = mybir.dt.bfloat16
CBRT2 = 2.0 ** (1.0 / 3.0)

USE_TILEPOS = True      # ping-pong weight tiles (0,0)/(0,64)
USE_LDW_SKIP = True     # reuse loaded weights for the 2nd column block


class _TC(tile.TileContext):
    """TileContext whose final drain splits sem waits to <=2 per instruction
    (walrus CTRL codegen rejects drains with too many sync waits)."""

    def _drain_and_barrier(self, tick_clock, wait_clock):
        from concourse.vector_clock import ScopedClock
        nc = self.nc
        drain_inst = nc.sync.drain()
        wait_clock.add_sem_waits(
            drain_inst.ins, ScopedClock({None: tick_clock.global_clock})
        )
        si = drain_inst.ins.sync_info
        waits = list(si.on_wait or [])
        MAXW = 1
        if len(waits) > MAXW:
            del si.on_wait[MAXW:]
            rest = waits[MAXW:]
            for i in range(0, len(rest), MAXW):
                d2 = nc.sync.drain()
                s2 = d2.ins.sync_info
                if s2 is None:
                    s2 = type(si)(on_wait=[], on_update=[])
                    d2.ins.sync_info = s2
                s2.on_wait.extend(rest[i:i + MAXW])
        nc.all_engine_barrier()
        popped = nc._tile_sem_poison_stack.pop()
        assert popped is self._sem_poison
        nc.clear_and_free_semaphores(list(self.sems.allocated().values()))
        nc.all_engine_barrier()


def _host_weights(coef, scale_base, scale_sp, bias):
    """bf16 lhsT weights [128, 21, 64]: groups 0-2 silu (rows 32*di+c,
    const/bias row 96 in group dj=1), groups 3+9*jc+k spline (rows 32*ml+c
    hold W_sp[o,c,k,4jc+ml]/12)."""
    W_sp = (scale_sp[..., None] * coef).astype(np.float32)   # (O, C, 9, M)
    wfull = np.zeros((128, NG, O), dtype=np.float32)
    for dj in range(3):
        for di in range(3):
            wfull[32 * di:32 * di + 32, dj, :] = (
                scale_base[:, :, 3 * di + dj].T)             # (c, o)
    wfull[96, 1, :] = bias
    for jc in range(2):
        for k in range(KH * KW):
            for ml in range(4):
                wfull[32 * ml:32 * ml + 32, 3 + 9 * jc + k, :] = (
                    W_sp[:, :, k, 4 * jc + ml].T / 12.0)
    return wfull.astype(ml_dtypes.bfloat16)


def _build_nc():
    nc = bass.Bass()
    # x (4 replica blocks) with 2 per-partition z-bias columns appended.
    x_d = nc.dram_tensor("xpad", [128, NPIX + 4], DT, kind="ExternalInput")
    w_d = nc.dram_tensor("wfull", [128, NG * O], DT, kind="ExternalInput")
    o_d = nc.dram_tensor("out", [O, H * W], mybir.dt.float32, kind="ExternalOutput")

    with _TC(nc) as tc:
        with tc.tile_pool(name="sb", bufs=1) as sb, \
             tc.tile_pool(name="eps", bufs=2) as eps, \
             tc.tile_pool(name="ps", bufs=2, space="PSUM") as ps:
            xb = sb.tile([128, NPIX + 4], DT)
            nc.sync.dma_start(out=xb[:, :], in_=x_d[:, :])
            xq = xb[:, 0:NPIX].rearrange("p (h w) -> p h w", h=HP)

            wf = sb.tile([128, NG * O], DT)
            nc.sync.dma_start(out=wf[:, :], in_=w_d[:, :])

            AF = mybir.ActivationFunctionType
            # ---- tiles. All matmul-facing writes go through DVE (silu staged
            # via sl) so matmuls carry one sem wait (walrus rejects >1 on MM).
            F = sb.tile([97, HP, WP], DT)
            sl = sb.tile([32, HP, WP], DT)
            scr = sb.tile([1, 1], DT)
            zt, Pt, Qt, P2t, P3t, Q2t, Q3t, chunks = [], [], [], [], [], [], [], []
            for jc in range(2):
                z = sb.tile([128, HP, WP], DT, tag=f"z{jc}")
                P = sb.tile([128, HP, WP], DT, tag=f"P{jc}")
                Q = sb.tile([128, HP, WP], DT, tag=f"Q{jc}")
                P2 = sb.tile([128, HP, WP], DT, tag=f"P2{jc}")
                P3 = sb.tile([128, HP, WP], DT, tag=f"P3{jc}")
                Q2 = sb.tile([128, HP, WP], DT, tag=f"Q2{jc}")
                Q3 = sb.tile([128, HP, WP], DT, tag=f"Q3{jc}")
                cb = sb.tile([128, HP, WP], DT, tag=f"cb{jc}")
                zt.append(z); Pt.append(P); Qt.append(Q); P2t.append(P2)
                P3t.append(P3); Q2t.append(Q2); Q3t.append(Q3); chunks.append(cb)

            # 1-element read of wf on DVE: its wait covers the weights DMA.
            nc.vector.tensor_copy(scr[:, :], wf[0:1, 0:1])
            nc.vector.memset(F[96:97, :, :], 1.0)

            # ---- features in two h-halves so lc=0 matmuls start early ----
            # half 0 covers grid rows [0,18) (lc=0 views), half 1 [18,34).
            for hh, (a, b) in enumerate(((0, 18), (18, 34))):
                sa, sb_ = (0, 20) if hh == 0 else (20, HP)
                for jc in range(2):
                    z, P, Q = zt[jc], Pt[jc], Qt[jc]
                    P2, P3, Q2, Q3, cb = (P2t[jc], P3t[jc], Q2t[jc], Q3t[jc],
                                          chunks[jc])
                    nc.scalar.activation(z[:, a:b, :], xq[:, a:b, :], AF.Abs,
                                         bias=xb[:, NPIX + jc:NPIX + jc + 1],
                                         scale=2.5)
                    nc.scalar.activation(P[:, a:b, :], z[:, a:b, :], AF.Relu,
                                         bias=xb[:, NPIX + 2:NPIX + 3],
                                         scale=-CBRT2)
                    nc.scalar.activation(Q[:, a:b, :], z[:, a:b, :], AF.Relu,
                                         bias=xb[:, NPIX + 3:NPIX + 4],
                                         scale=-2.0)
                    nc.vector.tensor_mul(P2[:, a:b, :], P[:, a:b, :], P[:, a:b, :])
                    nc.vector.tensor_mul(P3[:, a:b, :], P2[:, a:b, :], P[:, a:b, :])
                    nc.vector.tensor_mul(Q2[:, a:b, :], Q[:, a:b, :], Q[:, a:b, :])
                    nc.vector.tensor_mul(Q3[:, a:b, :], Q2[:, a:b, :], Q[:, a:b, :])
                    nc.vector.tensor_sub(cb[:, a:b, :], P3[:, a:b, :], Q3[:, a:b, :])
                    if jc == 0:
                        # silu half for F: rows [sa, sb_) of the padded grid
                        nc.scalar.activation(sl[:, sa:sb_, :], xq[0:32, sa:sb_, :],
                                             AF.Silu)
                        # F rows are only read for grid rows < 32 (view
                        # windows [0,16) and [16,32)): clamp the copies.
                        bc = min(b, 32)
                        nc.vector.tensor_copy(F[0:32, a:bc, :], sl[:, a:bc, :])
                        nc.vector.tensor_copy(F[32:64, a:bc, :],
                                              sl[:, a + 1:bc + 1, :])
                        nc.vector.tensor_copy(F[64:96, a:bc, :],
                                              sl[:, a + 2:bc + 2, :])

            # ---- 42 accumulating matmuls + epilogue ----
            psum = []
            for lc in range(2):
                pst = ps.tile([128, 512], mybir.dt.float32, tag=f"ps{lc}")
                psum.append(pst)
            # lc outer: all 21 groups for block 0 (h-half 0 features), then
            # block 1. Group order cb0, F, cb1 matches feature readiness.
            gorder = list(range(3, 12)) + [0, 1, 2] + list(range(12, NG))
            for lc in range(2):
                for gi, g in enumerate(gorder):
                    pos = (gi % 2) * 64 if USE_TILEPOS else 0
                    if g < 3:
                        rows, dj = 97, g
                        rhs = F[0:rows, lc * 16: lc * 16 + 16, dj: dj + 32]
                    else:
                        jc, k = (g - 3) // 9, (g - 3) % 9
                        rows, ik, jk = 128, k // 3, k % 3
                        rhs = chunks[jc][0:rows,
                                         lc * 16 + ik: lc * 16 + ik + 16,
                                         jk: jk + 32]
                    start = gi < (2 if USE_TILEPOS else 1)
                    stop = gi >= NG - (2 if USE_TILEPOS else 1)
                    nc.tensor.matmul(
                        psum[lc][pos: pos + O, :], wf[0:rows, O * g: O * g + O],
                        rhs, start=start, stop=stop,
                        tile_position=(0, pos) if USE_TILEPOS else None)
            for lc in range(2):
                ot = eps.tile([O, 512], mybir.dt.float32, tag="ot")
                if USE_TILEPOS:
                    # DVE-only epilogue (one PSUM operand per op): copy the lo
                    # half (finishes last, so its PE wait covers the hi half),
                    # then the add carries no waits.
                    oh = eps.tile([O, 512], mybir.dt.float32, tag="oh")
                    nc.vector.tensor_copy(oh[:, :], psum[lc][0:O, :])
                    nc.vector.tensor_add(ot[:, :], psum[lc][O:2 * O, :], oh[:, :])
                else:
                    nc.vector.tensor_copy(ot[:, :], psum[lc][0:O, :])
                nc.sync.dma_start(out=o_d[:, 512 * lc: 512 * (lc + 1)],
                                  in_=ot[:, :])
    return nc


_BVEC = np.zeros((128, 4), dtype=np.float32)
for _jc in range(2):
    for _ml in range(4):
        _BVEC[32 * _ml:32 * _ml + 32, _jc] = 3.5 - (4 * _jc + _ml)
_BVEC[:, 2] = 2.0 * CBRT2
_BVEC[:, 3] = 2.0

_NC_CACHE = {}


def _get_nc():
    if "nc" not in _NC_CACHE:
        _NC_CACHE["nc"] = _build_nc()
    return _NC_CACHE["nc"]


def _run(x, coef, scale_base, scale_sp, bias, trace=False):
    nc = _get_nc()
    kw = {}
    if trace:
        import os
        td = os.environ.get("KAN_TRACE_DIR")
        if td:
            os.makedirs(td, exist_ok=True)
            kw["tmpdir"] = td
    wfull = _host_weights(np.asarray(coef), np.asarray(scale_base),
                          np.asarray(scale_sp), np.asarray(bias))
    wflat = np.ascontiguousarray(wfull.reshape(128, -1))
    x = np.asarray(x)
    in_maps = []
    for b in range(B):
        xpad = np.zeros((C, HP, WP), dtype=np.float32)
        xpad[:, 1:1 + H, 1:1 + W] = x[b]
        x4 = np.tile(xpad.reshape(C, NPIX), (4, 1))
        xb = np.concatenate([x4, _BVEC], axis=1).astype(ml_dtypes.bfloat16)
        in_maps.append({"xpad": np.ascontiguousarray(xb), "wfull": wflat})
    res = run_bass_kernel_spmd(nc, in_maps, core_ids=list(range(B)), trace=trace,
                               **kw)
    out = np.stack([res.results[b]["out"].reshape(O, H, W) for b in range(B)])
    return out, res


def kernel(x, coef, scale_base, scale_sp, bias):
    out, _ = _run(x, coef, scale_base, scale_sp, bias, trace=False)
    return out


def kernel_traced(x, coef, scale_base, scale_sp, bias):
    # test.py injects the NTFF hook (antenv.axon_hooks) before importing us;
    # if absent, bass_utils degrades to untraced gracefully.
    out, res = _run(x, coef, scale_base, scale_sp, bias, trace=True)
    return out, res
